# revision 34
# baseline (speedup 1.0000x reference)
"""Distributed GCN (3x GCNConv + global_max_pool + MLP head) on 8 Trainium2
NeuronCores via concourse Bass/Tile SPMD. Graph-parallel: 8 graphs/core, node
rows block-packed per core. Aggregation = one-hot PE segment-sum in PSUM with
host-precomputed norm-folded S tiles (bf16, deduped by (src, dst-block));
self-loops via a per-block diagonal matmul against SBUF-resident bf16
prev-layer tiles (no gather); conv1's edge rows are host-gathered and DMA'd
as a plain param; conv2/3 rows gathered on-device via per-BLOCK batched SWDGE
indirect DMA (1024 rows/instruction) from fp8e4m3 h tables exchanged with
chunked 8-rank AllGathers that overlap the producing conv's compute. Pool:
batched one-hot gather + PE transpose + reduce_max; per-core MLP head. Host
only shards/packs/unshards.
"""
import sys
sys.path.insert(0, "/opt/trn_rl_repo")
import numpy as np

N = 20000
E = 160000
G = 64
IN = 128
HID = 512
ACTD = 32
NC = 8
RPC = 2560           # rows per core
BPC = 20             # blocks per core
NB = NC * BPC        # 160 global blocks
NPAD = NC * RPC      # 20480
NAG = 2              # AllGather chunks per layer
BPAG = BPC // NAG    # blocks per AG chunk (10)
CR = BPAG * 128      # rows per AG chunk per core (1280)


def _colmajor(a):
    """[M] -> [128, M/128]: idx i at (i%128, i//128), int32."""
    return a.reshape(-1, 128).T.astype(np.int32).copy()


def _wrap16(a):
    """[M] -> [128, M/16] int16 for dma_gather: idx i at (i%16, i//16),
    16-partition pattern replicated to all 128 partitions."""
    assert a.max() < 32768 and a.min() >= 0
    w = a.reshape(-1, 16).T.astype(np.int16)
    return np.tile(w, (8, 1)).copy()


def prep(inputs):
    import ml_dtypes
    bf16 = ml_dtypes.bfloat16
    f8 = ml_dtypes.float8_e4m3

    src = np.asarray(inputs["edge_index"][0], dtype=np.int64)
    dst = np.asarray(inputs["edge_index"][1], dtype=np.int64)
    batch = np.asarray(inputs["batch"], dtype=np.int64)
    tree_x = np.asarray(inputs["tree_x"], dtype=np.float32)

    deg = np.bincount(dst, minlength=N).astype(np.float64) + 1.0  # incl self-loop
    dinv = (1.0 / np.sqrt(deg)).astype(np.float32)

    gsizes = np.bincount(batch, minlength=G)
    # graphs -> cores: 8 consecutive per core; LPT fallback if any group > RPC
    groups = [list(range(8 * k, 8 * k + 8)) for k in range(NC)]
    if max(int(gsizes[g].sum()) for g in groups) > RPC:
        order = np.argsort(-gsizes)
        loads = [0] * NC
        counts = [0] * NC
        groups = [[] for _ in range(NC)]
        for g in order:
            k = min(range(NC), key=lambda i: (loads[i] if counts[i] < 8 else 1 << 60))
            groups[k].append(int(g))
            loads[k] += int(gsizes[g])
            counts[k] += 1
        assert max(loads) <= RPC, f"graph groups do not fit: {loads}"

    graph_core = np.zeros(G, dtype=np.int64)
    graph_slot = np.zeros(G, dtype=np.int64)
    for k in range(NC):
        for j, g in enumerate(groups[k]):
            graph_core[g] = k
            graph_slot[g] = j

    # --- node -> (core, block, slot) balancing per-core block edge loads ---
    import heapq
    newid = np.full(N, -1, dtype=np.int64)
    node_core = graph_core[batch]
    indeg = (deg - 1.0)  # in-edges excl self-loop drive gather load
    for k in range(NC):
        nodes = np.where(node_core == k)[0]
        nodes = nodes[np.argsort(-indeg[nodes], kind="stable")]
        heap = [(0.0, 0, b) for b in range(BPC)]  # (load, count, block)
        heapq.heapify(heap)
        for n in nodes:
            load, cnt, b = heapq.heappop(heap)
            newid[n] = k * RPC + b * 128 + cnt
            cnt += 1
            load += indeg[n]
            if cnt < 128:
                heapq.heappush(heap, (load, cnt, b))

    # --- edges (NO self-loops) grouped by dst block, deduped by (src, blk) ---
    norm_e = (dinv[src] * dinv[dst]).astype(np.float64)
    nd = newid[dst]
    blk = nd // 128
    # dedupe: one gathered row per (blk, src); S gets one entry per (src, dst)
    # pair (norms of parallel edges summed via np.add.at)
    key = blk * N + src
    uniq, inv = np.unique(key, return_inverse=True)
    n_uniq = len(uniq)
    u_blk = (uniq // N).astype(np.int64)
    u_src = (uniq % N).astype(np.int64)
    counts = np.bincount(u_blk, minlength=NB)
    T_B = int(np.ceil(counts.max() / 128.0))
    TPC = BPC * T_B
    cap = T_B * 128

    # single-AllGather table position: hf row = k*RPC + local = newid
    tblpos = newid
    TE = 0
    # slot of each unique (blk, src) within its block (uniq is blk-sorted)
    starts = np.concatenate([[0], np.cumsum(counts)])
    slot = np.arange(n_uniq) - starts[u_blk]
    # per-edge: S_all[blk, slot[uniq(e)], dst_rel(e)] += norm_e
    S_all = np.zeros((NB, cap, 128), dtype=np.float64)
    np.add.at(S_all, (blk, slot[inv], nd % 128), norm_e)

    src_rows = np.zeros((NB, cap), dtype=np.int64)   # gather ids (pad 0)
    src_rows[u_blk, slot] = u_src                    # original ids (for x/l1)

    # --- pooling row lists (pad = graph's own first row: max-safe) ---
    SLOTS_G = int(np.ceil(gsizes.max() / 128.0))
    pool_rows = np.zeros((NC, 8, SLOTS_G * 128), dtype=np.int64)
    for g in range(G):
        k, j = graph_core[g], graph_slot[g]
        rows = newid[np.where(batch == g)[0]] - k * RPC
        assert rows.min() >= 0 and rows.max() < RPC
        pool_rows[k, j, :] = rows[0]
        pool_rows[k, j, : len(rows)] = rows

    # --- per-core packed arrays ---
    # x in block layout (newid order); empty slots zero
    x_blocks = np.zeros((NPAD, IN), dtype=np.float32)
    x_blocks[newid] = tree_x
    dinv_rows = np.zeros(NPAD, dtype=np.float32)
    dinv_rows[newid] = dinv

    wkeys = ["W1", "b1", "W2", "b2", "W3", "b3", "Wf1", "bf1", "Wf2", "bf2",
             "Wf3", "bf3", "Wo", "bo"]
    weights = {k: np.asarray(inputs[k], dtype=np.float32) for k in wkeys}


    in_maps = []
    for k in range(NC):
        bsl = slice(k * BPC, (k + 1) * BPC)
        # S: [block, e, d] -> [p=e%128, gt, d] -> [128, TPC*128]
        S_k = S_all[bsl].reshape(TPC, 128, 128).transpose(1, 0, 2)
        # D: [128, BPC*128]; D[p, b*128+d] = (p==d) * dinv^2 of row (b,p)
        D_k = np.zeros((128, BPC * 128), dtype=np.float32)
        dv = dinv_rows[k * RPC:(k + 1) * RPC].reshape(BPC, 128)
        for b in range(BPC):
            D_k[np.arange(128), b * 128 + np.arange(128)] = dv[b] ** 2
        # g1: host-gathered x rows per edge slot: [p, gt, IN]
        g1_k = tree_x[src_rows[bsl].reshape(TPC, 128)]         # [gt, p, IN]
        g1_k = np.ascontiguousarray(g1_k.transpose(1, 0, 2))   # [p, gt, IN]
        m = {
            "S": np.ascontiguousarray(S_k.reshape(128, TPC * 128)).astype(bf16),
            "D": D_k.astype(bf16),
            "g1": g1_k.reshape(128, TPC * IN).astype(bf16),
            "x_own": x_blocks[k * RPC:(k + 1) * RPC].reshape(
                BPC, 128, IN).transpose(1, 0, 2).reshape(128, BPC * IN).astype(bf16),
            "idx23": _wrap16(tblpos[src_rows[bsl].reshape(-1)]),
            "pidx": _wrap16(pool_rows[k].reshape(-1)),
            "W1": weights["W1"].astype(bf16),
            "W2": weights["W2"].astype(bf16),
            "W3": weights["W3"].astype(bf16),
            "b1": weights["b1"][None, :].astype(bf16),
            "b2": weights["b2"][None, :].astype(bf16),
            "b3": weights["b3"][None, :].astype(bf16),
            "Wf1": weights["Wf1"], "Wf2": weights["Wf2"], "Wf3": weights["Wf3"],
            "bf1": weights["bf1"].reshape(4, 128).T.copy(),
            "bf2": weights["bf2"].reshape(4, 128).T.copy(),
            "bf3": weights["bf3"].reshape(4, 128).T.copy(),
            "Wo": weights["Wo"], "bo": weights["bo"][:, None],
        }
        in_maps.append(m)

    meta = dict(T_B=T_B, TPC=TPC, SLOTS_G=SLOTS_G, TE=TE, groups=groups,
                newid=newid, dinv=dinv, graph_core=graph_core,
                graph_slot=graph_slot)
    return in_maps, meta


def assemble_output(core_outs, meta):
    """core_outs: list of 8 arrays [ACTD, 8] -> full [64, ACTD]."""
    out = np.zeros((G, ACTD), dtype=np.float32)
    for k in range(NC):
        for j, g in enumerate(meta["groups"][k]):
            out[g] = core_outs[k][:, j]
    return out


from contextlib import ExitStack
import concourse.bass as bass
import concourse.bacc as bacc
import concourse.mybir as mybir
import concourse.tile as tile
from concourse.masks import make_identity

I32 = mybir.dt.int32
I16 = mybir.dt.int16
F32 = mybir.dt.float32
BF16 = mybir.dt.bfloat16
F8E4 = mybir.dt.float8e4
RELU = mybir.ActivationFunctionType.Relu
COPY = mybir.ActivationFunctionType.Copy


def build(T_B, SLOTS_G, TE=3):
    TPC = BPC * T_B
    TL = T_B - TE

    nc = bacc.Bacc("TRN2", num_devices=NC, num_swdge_queues=4,
                   dynamic_dma_scratch_size=32768)
    d = {}

    def param(name, shape, dt=F32):
        d[name] = nc.declare_dram_parameter(name, shape, dt, isOutput=False)

    param("S", [128, TPC * 128], BF16)
    param("D", [128, BPC * 128], BF16)
    param("g1", [128, TPC * IN], BF16)
    param("x_own", [128, BPC * IN], BF16)
    param("idx23", [128, TPC * 8], I16)          # TPC*128/16 cols
    param("pidx", [128, 8 * SLOTS_G * 8], I16)   # 8*SLOTS_G*128/16 cols
    param("W1", [IN, HID], BF16)
    param("W2", [HID, HID], BF16)
    param("W3", [HID, HID], BF16)
    for b in ["b1", "b2", "b3"]:
        param(b, [1, HID], BF16)
    for w in ["Wf1", "Wf2", "Wf3"]:
        param(w, [HID, HID])
    for b in ["bf1", "bf2", "bf3"]:
        param(b, [128, 4])
    param("Wo", [HID, ACTD])
    param("bo", [ACTD, 1])
    out = nc.declare_dram_parameter("out", [ACTD, 8], F32, isOutput=True)

    with tile.TileContext(nc) as tc, ExitStack() as ctx:
        cpool = ctx.enter_context(tc.tile_pool(name="const", bufs=1))
        dram = ctx.enter_context(tc.tile_pool(name="dram", bufs=1, space="DRAM"))
        gpool = ctx.enter_context(tc.tile_pool(name="gather", bufs=8))
        apool = ctx.enter_context(tc.tile_pool(name="agg", bufs=3))
        h8pool = ctx.enter_context(tc.tile_pool(name="h8", bufs=3))
        ppool_u = ctx.enter_context(tc.tile_pool(name="psum_u", bufs=3, space="PSUM"))
        ppool_t = ctx.enter_context(tc.tile_pool(name="psum_t", bufs=2, space="PSUM"))
        ppool_d = ctx.enter_context(tc.tile_pool(name="psum_d", bufs=2, space="PSUM"))

        # ---- DRAM intermediates ----
        h8a = dram.tile([RPC, HID], F8E4, name="h8a")       # conv1 out, fp8
        h8b = dram.tile([RPC, HID], F8E4, name="h8b")       # conv2 out, fp8
        hf1 = dram.tile([NPAD, HID], F8E4, addr_space="Shared", name="hf1")
        hf2 = dram.tile([NPAD, HID], F8E4, addr_space="Shared", name="hf2")

        # ---- constants to SBUF ----
        _ldq = [nc.sync, nc.scalar]
        _ldn = [0]

        def load(name, shape, dt=F32):
            t = cpool.tile(shape, dt, name=name)
            _ldq[_ldn[0] % 2].dma_start(out=t[:], in_=d[name][:])
            _ldn[0] += 1
            return t

        idx23 = load("idx23", [128, TPC * 8], I16)
        pidx = load("pidx", [128, 8 * SLOTS_G * 8], I16)
        Dsb = load("D", [128, BPC * 128], BF16)
        x_own = load("x_own", [128, BPC, IN], BF16)
        brows = {l: load(f"b{l}", [1, HID], BF16) for l in (1, 2, 3)}
        bfs = {f: load(f"bf{f}", [128, 4]) for f in (1, 2, 3)}
        bo = load("bo", [ACTD, 1])

        # S cache tiles: DMAs issued inside conv1's loop (interleaved with g1)
        Ssb = [cpool.tile([128, T_B * 128], BF16, name=f"S{b}")
               for b in range(BPC)]

        W1sb = load("W1", [128, HID], BF16)

        ident = cpool.tile([128, 128], BF16, name="ident")
        make_identity(nc, ident[:])
        ones_f32 = cpool.tile([1, 128], F32, name="ones_f32")
        nc.vector.memset(ones_f32[:], 1.0)
        ones_bf = cpool.tile([1, 128], BF16, name="ones_bf")
        nc.vector.tensor_copy(out=ones_bf[:], in_=ones_f32[:])

        # resident prev-layer tiles (conv output lives here; diag term reads it)
        hres = cpool.tile([128, BPC, HID], BF16, name="hres")

        groups8 = [list(range(NC))]


        # ---- one GCN conv layer ----
        def conv(l, src_dram, elem, Wt, brow, last, h8_dram=None, hf_out=None):
            nch = elem // 128
            for b in range(BPC):
                g = gpool.tile([128, T_B, elem],
                               BF16 if l == 1 else F8E4, name="g", tag="g")
                if l == 1:
                    # S on the scalar HWDGE queue, g1 on sync: 2x load rate
                    nc.scalar.dma_start(
                        out=Ssb[b][:],
                        in_=d["S"][:, b * T_B * 128:(b + 1) * T_B * 128])
                    nc.sync.dma_start(
                        out=g[:],
                        in_=d["g1"][:, b * T_B * elem:(b + 1) * T_B * elem]
                        .rearrange("p (t e) -> p t e", e=elem))
                else:
                    # two half-block SWDGE gathers on rotating queues:
                    # finer grain hides HBM random-read latency better
                    CW = T_B * 8      # idx cols per block (T_B*128/16)
                    TH = T_B // 2
                    for hh in range(2):
                        nc.gpsimd.dma_gather(
                            out_ap=g[:, hh * TH:(hh + 1) * TH, :],
                            in_ap=src_dram[:],
                            idxs_ap=idx23[:, b * CW + hh * TH * 8:
                                          b * CW + (hh + 1) * TH * 8],
                            num_idxs=TH * 128, num_idxs_reg=TH * 128,
                            elem_size=HID, queue_num=(2 * b + hh) % 4)
                u = ppool_u.tile([128, elem], F32, name="u", tag="u")
                for t in range(T_B):
                    nc.tensor.matmul(
                        u[:], lhsT=Ssb[b][:, t * 128:(t + 1) * 128],
                        rhs=g[:, t, :], start=(t == 0), stop=False)
                # self-loop diagonal term
                hprev = x_own[:, b, :] if l == 1 else hres[:, b, :]
                nc.tensor.matmul(
                    u[:], lhsT=Dsb[:, b * 128:(b + 1) * 128],
                    rhs=hprev, start=False, stop=True)
                agg = apool.tile([128, elem], BF16, name="agg", tag="agg")
                nc.vector.tensor_copy(out=agg[:], in_=u[:])
                tp = ppool_t.tile([128, elem], BF16, name="tp", tag="tp")
                for c in range(nch):
                    nc.tensor.transpose(tp[:, c * 128:(c + 1) * 128],
                                        agg[:, c * 128:(c + 1) * 128], ident[:])
                aggT = apool.tile([128, elem], BF16, name="aggT", tag="aggT")
                nc.vector.tensor_copy(out=aggT[:], in_=tp[:])
                hp = ppool_d.tile([128, HID], F32, name="hp", tag="hp")
                for c in range(nch):
                    Wc = Wt[:, c, :] if nch > 1 else Wt[:, :]
                    nc.tensor.matmul(hp[:], lhsT=aggT[:, c * 128:(c + 1) * 128],
                                     rhs=Wc, start=(c == 0), stop=False)
                nc.tensor.matmul(hp[:], lhsT=ones_bf[:, :], rhs=brow[:, :],
                                 start=False, stop=True)
                nc.scalar.activation(out=hres[:, b, :], in_=hp[:], func=RELU)
                if not last:
                    h8t = h8pool.tile([128, HID], F8E4, name="h8t", tag="h8t")
                    # second RELU straight from PSUM with fp8 output: keeps
                    # the bf16->fp8 downcast off the vector queue (the 2-5us
                    # DVE CASTs were serializing ahead of the aggT copies)
                    nc.scalar.activation(out=h8t[:], in_=hp[:], func=RELU)
                    nc.sync.dma_start(out=h8_dram[b * 128:(b + 1) * 128, :],
                                      in_=h8t[:])
            if not last:
                nc.gpsimd.collective_compute(
                    "AllGather", mybir.AluOpType.bypass,
                    replica_groups=groups8,
                    ins=[h8_dram[:]], outs=[hf_out[:]])

        with nc.named_scope("conv1"):
            conv(1, None, IN, W1sb, brows[1], last=False, h8_dram=h8a, hf_out=hf1)
        # late weight loads: overlap with conv1/ag window
        Wsb = {}
        for l, wn in ((2, "W2"), (3, "W3")):
            t = cpool.tile([128, 4, HID], BF16, name=wn + "sb")
            for c in range(4):
                nc.sync.dma_start(out=t[:, c, :], in_=d[wn][c * 128:(c + 1) * 128, :])
            Wsb[l] = t
        Wfsb = {}
        for f in (1, 2, 3):
            t = cpool.tile([128, 4, HID], F32, name=f"Wf{f}sb")
            for c in range(4):
                nc.sync.dma_start(out=t[:, c, :], in_=d[f"Wf{f}"][c * 128:(c + 1) * 128, :])
            Wfsb[f] = t
        Wosb = cpool.tile([128, 4, ACTD], F32, name="Wosb")
        for c in range(4):
            nc.sync.dma_start(out=Wosb[:, c, :], in_=d["Wo"][c * 128:(c + 1) * 128, :])
        with nc.named_scope("conv2"):
            conv(2, hf1, HID, Wsb[2], brows[2], last=False, h8_dram=h8b, hf_out=hf2)
        with nc.named_scope("conv3"):
            conv(3, hf2, HID, Wsb[3], brows[3], last=True)

        # ---- pooling: SBUF-source transpose-gather from hres + reduce_max ----
        nc.enter_named_scope("pool", False)
        PN = SLOTS_G * 128   # gathered rows per graph (padded w/ first row)
        PW = PN // 16        # idx cols per graph
        pooled = cpool.tile([128, 32], F32, name="pooled")
        for j in range(8):
            pt = apool.tile([128, 4, PN], BF16, name="pt", tag="pt")
            nc.gpsimd.dma_gather(
                out_ap=pt[:], in_ap=hres[:],
                idxs_ap=pidx[:, j * PW:(j + 1) * PW],
                num_idxs=PN, num_idxs_reg=PN,
                elem_size=HID, transpose=True,
                sbuf_tokens_per_rank=128,
                sbuf_free_dim_per_rank=HID * 2, queue_num=j % 4)
            for c in range(4):
                nc.vector.reduce_max(
                    out=pooled[:, c * 8 + j:c * 8 + j + 1], in_=pt[:, c, :],
                    axis=mybir.AxisListType.X)

        # ---- MLP head (per-core on its 8 graphs, fp32) ----
        xcur = pooled
        for f in (1, 2, 3):
            hp2 = ppool_d.tile([128, 32], F32, name="hp2", tag="hp")
            for co in range(4):
                for ci in range(4):
                    nc.tensor.matmul(
                        hp2[:, co * 8:(co + 1) * 8],
                        lhsT=Wfsb[f][:, ci, co * 128:(co + 1) * 128],
                        rhs=xcur[:, ci * 8:(ci + 1) * 8],
                        start=(ci == 0), stop=(ci == 3))
            xnext = cpool.tile([128, 32], F32, name=f"x{f}")
            for co in range(4):
                nc.scalar.activation(out=xnext[:, co * 8:(co + 1) * 8],
                                     in_=hp2[:, co * 8:(co + 1) * 8], func=RELU,
                                     bias=bfs[f][:, co:co + 1])
            xcur = xnext
        po = ppool_d.tile([ACTD, 8], F32, name="po", tag="hp")
        for ci in range(4):
            nc.tensor.matmul(po[:], lhsT=Wosb[:, ci, :],
                             rhs=xcur[:, ci * 8:(ci + 1) * 8],
                             start=(ci == 0), stop=(ci == 3))
        nc.leave_named_scope("pool", None, False)
        osb = cpool.tile([ACTD, 8], F32, name="osb")
        nc.vector.tensor_scalar_add(out=osb[:], in0=po[:], scalar1=bo[:, 0:1])
        nc.sync.dma_start(out=out[:], in_=osb[:])

    nc.compile()
    return nc


_CACHE = {}


def kernel(**inputs) -> np.ndarray:
    in_maps, meta = prep(inputs)
    key = (meta["T_B"], meta["SLOTS_G"], meta["TE"])
    if key not in _CACHE:
        _CACHE[key] = build(meta["T_B"], meta["SLOTS_G"], meta["TE"])
    nc = _CACHE[key]
    from concourse.bass_utils import run_bass_kernel_spmd
    res = run_bass_kernel_spmd(nc, in_maps, list(range(NC)))
    core_outs = [res.results[k]["out"] for k in range(NC)]
    return assemble_output(core_outs, meta)
